# revision 20
# baseline (speedup 1.0000x reference)
"""FALCON ObjectSomeValuesFrom forward kernel for Trainium2 (8 NeuronCores).

Math (reference):
    e_all = concat(e_table, anon_e_emb)            # [n, d], n=1024, d=128
    Wl, Wr = W0[:, :d], W0[:, d:]
    c_fs  = sigmoid(leaky(c@Wl.T + e_all@Wr.T + b0) @ W1 + b1)        # [n]
    left  = (e_all + r) @ Wl.T ; rightp = e_all @ Wr.T + b0
    z_ij  = leaky(left_i + rightp_j) @ W1                              # [n, n]
    out_i = max_j sigmoid(z_ij + b1) * c_fs[j]

Trick: leaky(x) = 0.1*x + 0.9*relu(x) (slope 0.1), so
    z_ij = 0.1*(lin_i + lin_j) + sum_k (0.9*W1_k) * relu(left_ik + rp_jk)
with lin_i = left_i@W1, lin_j = rightp_j@W1.

Device mapping (per core, 128 "i" rows, all 1024 "j" columns):
  - relu tile A_i [128(k), 1024(j)] bf16 generated by DVE (tensor_scalar
    add+max chain, cols < JD) and ACT (activation Relu with per-partition
    bias, cols >= JD).
  - PE contracts A_i with a one-hot stationary operand so row i of the
    PSUM [128, 512] accumulators receives the relu part of z_i.  i's are
    processed in blocks of 4 mapped to the four PE column strips
    (tile_position=(0,32g), strip g owns out partitions [32g,32g+32)),
    so the 4 matmuls of a block stream concurrently through the array.
  - 0.1*lin_j is folded into the same PSUM accumulation with one
    broadcast-weight matmul per bank: lhsT = (0.1*W1) replicated in all
    128 columns, rhs = rbT (bf16 rightp).
  - 0.1*lin_i + b1 is the per-partition sigmoid bias; c_fs comes from an
    identical relu-contraction with the c-embedding bias, broadcast
    across partitions with a K=1 ones matmul.

Sharding: i-rows (left operand rows) split across 8 cores; e_table,
weights and c/r embeddings replicated; the final max over j is local.
"""

import numpy as np
import ml_dtypes

N = 1024
D = 128
NCORES = 8
IPC = N // NCORES  # i rows per core = 128
JD = 832  # relu-tile columns produced by DVE
JA = 192  # columns produced by ACT (after JD)

_PROGRAM_CACHE: dict = {}

# fp32 input pack layout (columns): cols[8] | e_myT[128] | w0T[256]
_FP_COLS = 8 + IPC + 2 * D
# bf16 input pack layout: e_allT[1024] | w1rep[128] | sbh[256] | wrT[128]
_BF_COLS = N + D + 2 * D + D


def _build_program(b1f: float):
    import concourse.bacc as bacc
    import concourse.mybir as mybir
    import concourse.tile as tile

    f32 = mybir.dt.float32
    bf16 = mybir.dt.bfloat16
    A_OP = mybir.AluOpType
    AF = mybir.ActivationFunctionType

    nc = bacc.Bacc(None, target_bir_lowering=False, name="falcon_fwd")

    d_fp = nc.dram_tensor("fp_pack", [D, _FP_COLS], f32, kind="ExternalInput")
    d_bf = nc.dram_tensor("bf_pack", [D, _BF_COLS], bf16, kind="ExternalInput")
    d_rows = nc.dram_tensor("rows", [1, D], f32, kind="ExternalInput")
    d_out = nc.dram_tensor("out", [IPC, 1], f32, kind="ExternalOutput")

    H = N // 2  # 512, PSUM bank free size

    with tile.TileContext(nc) as tc:
        with (
            tc.tile_pool(name="const", bufs=1) as const,
            tc.tile_pool(name="big", bufs=1) as big,
            tc.tile_pool(name="work", bufs=10) as work,
            tc.tile_pool(name="ps", bufs=4, space="PSUM") as ps,
            tc.tile_pool(name="psz", bufs=2, space="PSUM") as psz,
        ):
            # ---- load inputs (two DMA issue queues, e_allT halves
            # early so the rightp matmuls start before loads finish) ---
            fp = const.tile([D, _FP_COLS], f32)
            bf = big.tile([D, _BF_COLS], bf16)
            rows_raw = const.tile([1, D], f32)
            nc.sync.dma_start(fp[:, 0:8], d_fp[:, 0:8])
            nc.sync.dma_start(bf[:, N:], d_bf[:, N:])  # weights tail
            nc.sync.dma_start(fp[:, 8:], d_fp[:, 8:])
            nc.sync.dma_start(bf[:, :N], d_bf[:, :N])
            nc.sync.dma_start(rows_raw[:], d_rows[:])

            # Funnel the scalar-pointer source through one DVE copy so
            # TensorScalarPtr ops downstream wait on one semaphore only.
            colsS = const.tile([D, 8], f32)
            nc.vector.tensor_copy(colsS[:], fp[:, 0:8])
            rowsS = const.tile([1, D], f32)
            nc.vector.tensor_copy(rowsS[:], rows_raw[:])

            w1c = colsS[:, 0:1]
            b0c = colsS[:, 2:3]
            rc = colsS[:, 3:4]
            cc = colsS[:, 4:5]
            emyT = fp[:, 8 : 8 + IPC]
            wlT = fp[:, 8 + IPC : 8 + IPC + D]
            eallT = bf[:, :N]
            w1rep = bf[:, N : N + D]
            sbh = bf[:, N + D : N + 3 * D]
            w101c = bf[:, N + 3 * D - 1 : N + 3 * D]  # 0.1*W1 col (last sbh col)
            wrT = bf[:, N + 3 * D :]  # bf16 Wr^T (host-cast)
            ones_row = rowsS[:, :]

            # ---- prologue -------------------------------------------
            # er_myT = e_myT + r  (broadcast r along i)
            er_myT = const.tile([D, IPC], f32)
            nc.vector.tensor_scalar(er_myT[:], emyT, rc, None, A_OP.add)
            # leftT (unscaled) for my rows: [d_out=128, i=128]
            left_ps = ps.tile([D, IPC], f32, tag="ps")
            nc.tensor.matmul(left_ps[:], wlT, er_myT[:], start=True, stop=True)
            leftT = const.tile([D, IPC], f32)
            nc.scalar.copy(leftT[:], left_ps[:])

            # rbT = bf16(rightp^T) = bf16(Wr@e_allT + b0), halves of j
            rbT = big.tile([D, N], bf16)
            for h in range(2):
                sl = slice(h * H, (h + 1) * H)
                rp_ps = ps.tile([D, H], f32, tag="ps")
                nc.tensor.matmul(
                    rp_ps[:], wrT, eallT[:, sl], start=True, stop=True
                )
                nc.vector.tensor_scalar(rbT[:, sl], rp_ps[:], b0c, None, A_OP.add)

            # lin_i column -> sigmoid bias vec = 0.1*lin_i + b1
            lini_ps = ps.tile([IPC, 1], f32, tag="ps")
            nc.tensor.matmul(lini_ps[:], leftT[:], w1c, start=True, stop=True)
            biasvec = const.tile([IPC, 1], f32)
            nc.vector.tensor_scalar(
                biasvec[:], lini_ps[:], 0.1, b1f, A_OP.mult, A_OP.add
            )

            # ---- c-branch: c_fs over all j --------------------------
            cl_ps = ps.tile([D, 1], f32, tag="ps")
            nc.tensor.matmul(cl_ps[:], wlT, cc, start=True, stop=True)
            cl = const.tile([D, 1], f32)
            nc.scalar.copy(cl[:], cl_ps[:])
            clw_ps = ps.tile([1, 1], f32, tag="ps")
            nc.tensor.matmul(clw_ps[:], cl[:], w1c, start=True, stop=True)
            bc = const.tile([1, 1], f32)
            nc.vector.tensor_scalar(bc[:], clw_ps[:], 0.1, b1f, A_OP.mult, A_OP.add)

            cfs_row = const.tile([1, N], f32)

            # ---- main loop: relu-part accumulation over my 128 rows -
            # (the c_fs partition-broadcast is emitted after block 1 so
            # its PE matmuls and DVE copies slot into engine idle time
            # instead of stalling the FIFO queues at loop start)
            cfsrep = big.tile([D, N], f32)
            z0 = psz.tile([D, H], f32, tag="z")
            z1 = psz.tile([D, H], f32, tag="z")
            NB = 32  # blocks; block b handles i in {b, 32+b, 64+b, 96+b}
            for b in range(NB):
                if b == 1:
                    # c-branch relu tiles + contraction, slotted here so
                    # their FIFO stalls overlap block-0 compute
                    for h in range(2):
                        sl = slice(h * H, (h + 1) * H)
                        Ac = work.tile([D, H], bf16, tag="Ac")
                        nc.scalar.activation(
                            Ac[:], rbT[:, sl], AF.Relu, bias=cl[:], scale=1.0
                        )
                        zc_ps = ps.tile([1, H], f32, tag="ps")
                        nc.tensor.matmul(
                            zc_ps[:], sbh[:, D : D + 1], Ac[:],
                            start=True, stop=False,
                        )
                        nc.tensor.matmul(
                            zc_ps[:], w101c, rbT[:, sl], start=False, stop=True
                        )
                        nc.scalar.activation(
                            cfs_row[:, sl], zc_ps[:], AF.Sigmoid,
                            bias=bc[:], scale=1.0,
                        )
                if b == 3:
                    for h in range(2):
                        sl = slice(h * H, (h + 1) * H)
                        cr_ps = ps.tile([D, H], f32, tag="ps")
                        nc.tensor.matmul(
                            cr_ps[:], ones_row, cfs_row[0:1, sl],
                            start=True, stop=True,
                        )
                        nc.vector.tensor_copy(cfsrep[:, sl], cr_ps[:])
                # One full [128, 1024] relu tile per i; 3 of the 4 block
                # rows from DVE, 1 from ACT (per-op fixed cost is paid
                # once per i instead of once per engine per i).
                tiles = []
                for g in range(4):
                    i = 32 * g + b
                    A = work.tile([D, N], bf16, tag="Av" if g < 3 else "As")
                    bias_i = leftT[:, i : i + 1]
                    if g < 3:
                        nc.vector.tensor_scalar(
                            A[:], rbT[:], bias_i, 0.0, A_OP.add, A_OP.max
                        )
                    else:
                        nc.scalar.activation(
                            A[:], rbT[:], AF.Relu, bias=bias_i, scale=1.0
                        )
                    tiles.append(A)
                w_b = sbh[:, D - b : D - b + 32]
                st = b == 0
                for g in range(4):
                    sl = slice(32 * g, 32 * g + 32)
                    nc.tensor.matmul(
                        z0[sl, :], w_b, tiles[g][:, :H], start=st, stop=False,
                        tile_position=(0, 32 * g), skip_group_check=True,
                    )
                for g in range(4):
                    sl = slice(32 * g, 32 * g + 32)
                    nc.tensor.matmul(
                        z1[sl, :], w_b, tiles[g][:, H:], start=st, stop=False,
                        tile_position=(0, 32 * g), skip_group_check=True,
                    )
                if b == 0:
                    # fold 0.1*lin_j into every row: lhsT has 0.1*W1 in
                    # all 128 columns, rhs = rightp (bf16).  Early: PSUM
                    # accumulation is order-independent.
                    nc.tensor.matmul(
                        z0[:], w1rep, rbT[:, :H], start=False, stop=False,
                        skip_group_check=True,
                    )
                    nc.tensor.matmul(
                        z1[:], w1rep, rbT[:, H:], start=False, stop=False,
                        skip_group_check=True,
                    )

            # ---- epilogue (pipelined per PSUM bank) ------------------
            rfs = big.tile([D, N], f32)
            prod = big.tile([D, N], f32)
            outc2 = const.tile([IPC, 2], f32)
            for h, zb in ((0, z0), (1, z1)):
                sl = slice(h * H, (h + 1) * H)
                nc.scalar.activation(
                    rfs[:, sl], zb[:], AF.Sigmoid, bias=biasvec[:], scale=1.0
                )
                nc.vector.tensor_tensor(
                    prod[:, sl], rfs[:, sl], cfsrep[:, sl], A_OP.mult
                )
                nc.vector.tensor_reduce(
                    outc2[:, h : h + 1],
                    prod[:, sl],
                    axis=mybir.AxisListType.X,
                    op=A_OP.max,
                )
            outc = const.tile([IPC, 1], f32)
            nc.vector.tensor_tensor(
                outc[:], outc2[:, 0:1], outc2[:, 1:2], A_OP.max
            )
            nc.gpsimd.dma_start(d_out[:], outc[:])

    return nc


def _host_prep(anon_e_emb, e_table, c_emb, r_emb, W0, b0, W1, b1):
    f = np.float32
    bft = ml_dtypes.bfloat16
    anon_e_emb = np.asarray(anon_e_emb, f)
    e_table = np.asarray(e_table, f)
    c_emb = np.asarray(c_emb, f)
    r_emb = np.asarray(r_emb, f)
    W0 = np.asarray(W0, f)
    b0 = np.asarray(b0, f)
    W1 = np.asarray(W1, f)
    b1 = np.asarray(b1, f)

    e_all = np.concatenate([e_table, anon_e_emb], axis=0)  # [N, D]
    e_allT = np.ascontiguousarray(e_all.T)  # [D, N]

    cols = np.zeros((D, 8), f)
    cols[:, 0] = W1
    cols[:, 2] = b0
    cols[:, 3] = r_emb
    cols[:, 4] = c_emb

    w0T = np.concatenate([W0[:, :D].T, W0[:, D:].T], axis=1)  # [wlT | wrT]

    bf_pack = np.zeros((D, _BF_COLS), bft)
    bf_pack[:, :N] = e_allT.astype(bft)
    bf_pack[:, N : N + D] = np.tile((0.1 * W1).astype(bft)[:, None], (1, D))
    # sbh: one-hot window buffer, col 128 = 0.9*W1; last col = 0.1*W1
    # (used as the lin_j column for the c-branch).
    sbh = np.zeros((D, 2 * D), bft)
    sbh[:, D] = (0.9 * W1).astype(bft)
    sbh[:, 2 * D - 1] = (0.1 * W1).astype(bft)
    bf_pack[:, N + D : N + 3 * D] = sbh
    bf_pack[:, N + 3 * D :] = W0[:, D:].T.astype(bft)

    rows = np.ones((1, D), f)
    b1f = float(b1[0])

    in_maps = []
    for c in range(NCORES):
        fp_pack = np.zeros((D, _FP_COLS), f)
        fp_pack[:, 0:8] = cols
        fp_pack[:, 8 : 8 + IPC] = e_allT[:, c * IPC : (c + 1) * IPC]
        fp_pack[:, 8 + IPC :] = w0T
        in_maps.append({"fp_pack": fp_pack, "bf_pack": bf_pack, "rows": rows})
    return in_maps, b1f


def _install_ntff_shim():
    """Provide antenv.axon_hooks (missing in this image) so that
    run_bass_kernel_spmd(trace=True) can collect NTFF profiles."""
    import sys
    import types

    if "antenv.axon_hooks" in sys.modules:
        return
    try:
        import antenv
        from trn_agent_boot.trn_boot import _ntff_profile_via_ctypes
    except ImportError:
        return
    mod = types.ModuleType("antenv.axon_hooks")
    state = {"hook": None}
    mod.set_axon_ntff_profile_hook = lambda h: state.__setitem__("hook", h)
    mod.get_axon_ntff_profile_hook = lambda: state["hook"]
    sys.modules["antenv.axon_hooks"] = mod
    antenv.axon_hooks = mod
    try:
        mod.set_axon_ntff_profile_hook(
            _ntff_profile_via_ctypes("/opt/axon/libaxon_pjrt.so")
        )
    except Exception:
        pass


def kernel_ex(inputs: dict, trace: bool = False):
    """Run on 8 NeuronCores; returns (out [N] float32, BassKernelResults)."""
    from concourse.bass_utils import run_bass_kernel_spmd

    if trace:
        _install_ntff_shim()

    in_maps, b1f = _host_prep(**inputs)
    key = (round(b1f, 10),)
    nc = _PROGRAM_CACHE.get(key)
    if nc is None:
        nc = _build_program(b1f)
        nc.finalize()
        _PROGRAM_CACHE[key] = nc

    res = run_bass_kernel_spmd(
        nc, in_maps, core_ids=list(range(NCORES)), trace=trace
    )
    out = np.concatenate(
        [
            np.asarray(res.results[c]["out"], np.float32).reshape(IPC)
            for c in range(NCORES)
        ]
    )
    return out, res


def kernel(**inputs) -> np.ndarray:
    out, _ = kernel_ex(inputs, trace=False)
    return out


# revision 21
# speedup vs baseline: 1.0005x; 1.0005x over previous
"""FALCON ObjectSomeValuesFrom forward kernel for Trainium2 (8 NeuronCores).

Math (reference):
    e_all = concat(e_table, anon_e_emb)            # [n, d], n=1024, d=128
    Wl, Wr = W0[:, :d], W0[:, d:]
    c_fs  = sigmoid(leaky(c@Wl.T + e_all@Wr.T + b0) @ W1 + b1)        # [n]
    left  = (e_all + r) @ Wl.T ; rightp = e_all @ Wr.T + b0
    z_ij  = leaky(left_i + rightp_j) @ W1                              # [n, n]
    out_i = max_j sigmoid(z_ij + b1) * c_fs[j]

Trick: leaky(x) = 0.1*x + 0.9*relu(x) (slope 0.1), so
    z_ij = 0.1*(lin_i + lin_j) + sum_k (0.9*W1_k) * relu(left_ik + rp_jk)
with lin_i = left_i@W1, lin_j = rightp_j@W1.

Device mapping (per core, 128 "i" rows, all 1024 "j" columns):
  - relu tile A_i [128(k), 1024(j)] bf16 generated by DVE (tensor_scalar
    add+max chain, cols < JD) and ACT (activation Relu with per-partition
    bias, cols >= JD).
  - PE contracts A_i with a one-hot stationary operand so row i of the
    PSUM [128, 512] accumulators receives the relu part of z_i.  i's are
    processed in blocks of 4 mapped to the four PE column strips
    (tile_position=(0,32g), strip g owns out partitions [32g,32g+32)),
    so the 4 matmuls of a block stream concurrently through the array.
  - 0.1*lin_j is folded into the same PSUM accumulation with one
    broadcast-weight matmul per bank: lhsT = (0.1*W1) replicated in all
    128 columns, rhs = rbT (bf16 rightp).
  - 0.1*lin_i + b1 is the per-partition sigmoid bias; c_fs comes from an
    identical relu-contraction with the c-embedding bias, broadcast
    across partitions with a K=1 ones matmul.

Sharding: i-rows (left operand rows) split across 8 cores; e_table,
weights and c/r embeddings replicated; the final max over j is local.
"""

import numpy as np
import ml_dtypes

N = 1024
D = 128
NCORES = 8
IPC = N // NCORES  # i rows per core = 128
JD = 832  # relu-tile columns produced by DVE
JA = 192  # columns produced by ACT (after JD)

_PROGRAM_CACHE: dict = {}

# fp32 input pack layout (columns): cols[8] | e_myT[128] | w0T[256]
_FP_COLS = 8 + IPC + 2 * D
# bf16 input pack layout: e_allT[1024] | w1rep[128] | sbh[256] | wrT[128]
_BF_COLS = N + D + 2 * D + D


def _build_program(b1f: float):
    import concourse.bacc as bacc
    import concourse.mybir as mybir
    import concourse.tile as tile

    f32 = mybir.dt.float32
    bf16 = mybir.dt.bfloat16
    A_OP = mybir.AluOpType
    AF = mybir.ActivationFunctionType

    nc = bacc.Bacc(None, target_bir_lowering=False, name="falcon_fwd")

    d_fp = nc.dram_tensor("fp_pack", [D, _FP_COLS], f32, kind="ExternalInput")
    d_bf = nc.dram_tensor("bf_pack", [D, _BF_COLS], bf16, kind="ExternalInput")
    d_rows = nc.dram_tensor("rows", [1, D], f32, kind="ExternalInput")
    d_out = nc.dram_tensor("out", [IPC, 1], f32, kind="ExternalOutput")

    H = N // 2  # 512, PSUM bank free size

    with tile.TileContext(nc) as tc:
        with (
            tc.tile_pool(name="const", bufs=1) as const,
            tc.tile_pool(name="big", bufs=1) as big,
            tc.tile_pool(name="work", bufs=10) as work,
            tc.tile_pool(name="ps", bufs=4, space="PSUM") as ps,
            tc.tile_pool(name="psz", bufs=2, space="PSUM") as psz,
        ):
            # ---- load inputs (two DMA issue queues, e_allT halves
            # early so the rightp matmuls start before loads finish) ---
            fp = const.tile([D, _FP_COLS], f32)
            bf = big.tile([D, _BF_COLS], bf16)
            rows_raw = const.tile([1, D], f32)
            nc.sync.dma_start(fp[:], d_fp[:])
            nc.sync.dma_start(bf[:, N:], d_bf[:, N:])  # weights tail
            nc.sync.dma_start(bf[:, :N], d_bf[:, :N])
            nc.sync.dma_start(rows_raw[:], d_rows[:])

            # Funnel the scalar-pointer source through one DVE copy so
            # TensorScalarPtr ops downstream wait on one semaphore only.
            colsS = const.tile([D, 8], f32)
            nc.vector.tensor_copy(colsS[:], fp[:, 0:8])
            rowsS = const.tile([1, D], f32)
            nc.vector.tensor_copy(rowsS[:], rows_raw[:])

            w1c = colsS[:, 0:1]
            b0c = colsS[:, 2:3]
            rc = colsS[:, 3:4]
            cc = colsS[:, 4:5]
            emyT = fp[:, 8 : 8 + IPC]
            wlT = fp[:, 8 + IPC : 8 + IPC + D]
            eallT = bf[:, :N]
            w1rep = bf[:, N : N + D]
            sbh = bf[:, N + D : N + 3 * D]
            w101c = bf[:, N + 3 * D - 1 : N + 3 * D]  # 0.1*W1 col (last sbh col)
            wrT = bf[:, N + 3 * D :]  # bf16 Wr^T (host-cast)
            ones_row = rowsS[:, :]

            # ---- prologue -------------------------------------------
            # er_myT = e_myT + r  (broadcast r along i)
            er_myT = const.tile([D, IPC], f32)
            nc.vector.tensor_scalar(er_myT[:], emyT, rc, None, A_OP.add)
            # leftT (unscaled) for my rows: [d_out=128, i=128]
            left_ps = ps.tile([D, IPC], f32, tag="ps")
            nc.tensor.matmul(left_ps[:], wlT, er_myT[:], start=True, stop=True)
            leftT = const.tile([D, IPC], f32)
            nc.scalar.copy(leftT[:], left_ps[:])

            # rbT = bf16(rightp^T) = bf16(Wr@e_allT + b0), halves of j
            rbT = big.tile([D, N], bf16)
            for h in range(2):
                sl = slice(h * H, (h + 1) * H)
                rp_ps = ps.tile([D, H], f32, tag="ps")
                nc.tensor.matmul(
                    rp_ps[:], wrT, eallT[:, sl], start=True, stop=True
                )
                nc.vector.tensor_scalar(rbT[:, sl], rp_ps[:], b0c, None, A_OP.add)

            # lin_i column -> sigmoid bias vec = 0.1*lin_i + b1
            lini_ps = ps.tile([IPC, 1], f32, tag="ps")
            nc.tensor.matmul(lini_ps[:], leftT[:], w1c, start=True, stop=True)
            biasvec = const.tile([IPC, 1], f32)
            nc.vector.tensor_scalar(
                biasvec[:], lini_ps[:], 0.1, b1f, A_OP.mult, A_OP.add
            )

            # ---- c-branch: c_fs over all j --------------------------
            cl_ps = ps.tile([D, 1], f32, tag="ps")
            nc.tensor.matmul(cl_ps[:], wlT, cc, start=True, stop=True)
            cl = const.tile([D, 1], f32)
            nc.scalar.copy(cl[:], cl_ps[:])
            clw_ps = ps.tile([1, 1], f32, tag="ps")
            nc.tensor.matmul(clw_ps[:], cl[:], w1c, start=True, stop=True)
            bc = const.tile([1, 1], f32)
            nc.vector.tensor_scalar(bc[:], clw_ps[:], 0.1, b1f, A_OP.mult, A_OP.add)

            cfs_row = const.tile([1, N], f32)

            # ---- main loop: relu-part accumulation over my 128 rows -
            # (the c_fs partition-broadcast is emitted after block 1 so
            # its PE matmuls and DVE copies slot into engine idle time
            # instead of stalling the FIFO queues at loop start)
            cfsrep = big.tile([D, N], f32)
            z0 = psz.tile([D, H], f32, tag="z")
            z1 = psz.tile([D, H], f32, tag="z")
            NB = 32  # blocks; block b handles i in {b, 32+b, 64+b, 96+b}
            for b in range(NB):
                if b == 1:
                    # c-branch relu tiles + contraction, slotted here so
                    # their FIFO stalls overlap block-0 compute
                    for h in range(2):
                        sl = slice(h * H, (h + 1) * H)
                        Ac = work.tile([D, H], bf16, tag="Ac")
                        nc.scalar.activation(
                            Ac[:], rbT[:, sl], AF.Relu, bias=cl[:], scale=1.0
                        )
                        zc_ps = ps.tile([1, H], f32, tag="ps")
                        nc.tensor.matmul(
                            zc_ps[:], sbh[:, D : D + 1], Ac[:],
                            start=True, stop=False,
                        )
                        nc.tensor.matmul(
                            zc_ps[:], w101c, rbT[:, sl], start=False, stop=True
                        )
                        nc.scalar.activation(
                            cfs_row[:, sl], zc_ps[:], AF.Sigmoid,
                            bias=bc[:], scale=1.0,
                        )
                if b == 3:
                    for h in range(2):
                        sl = slice(h * H, (h + 1) * H)
                        cr_ps = ps.tile([D, H], f32, tag="ps")
                        nc.tensor.matmul(
                            cr_ps[:], ones_row, cfs_row[0:1, sl],
                            start=True, stop=True,
                        )
                        nc.vector.tensor_copy(cfsrep[:, sl], cr_ps[:])
                # One full [128, 1024] relu tile per i; 3 of the 4 block
                # rows from DVE, 1 from ACT (per-op fixed cost is paid
                # once per i instead of once per engine per i).
                tiles = []
                n_dve = 2 if b % 8 == 7 else 3  # 92 DVE : 36 ACT rows
                for g in range(4):
                    i = 32 * g + b
                    A = work.tile([D, N], bf16, tag="Av" if g < n_dve else "As")
                    bias_i = leftT[:, i : i + 1]
                    if g < n_dve:
                        nc.vector.tensor_scalar(
                            A[:], rbT[:], bias_i, 0.0, A_OP.add, A_OP.max
                        )
                    else:
                        nc.scalar.activation(
                            A[:], rbT[:], AF.Relu, bias=bias_i, scale=1.0
                        )
                    tiles.append(A)
                w_b = sbh[:, D - b : D - b + 32]
                st = b == 0
                for g in range(4):
                    sl = slice(32 * g, 32 * g + 32)
                    nc.tensor.matmul(
                        z0[sl, :], w_b, tiles[g][:, :H], start=st, stop=False,
                        tile_position=(0, 32 * g), skip_group_check=True,
                    )
                for g in range(4):
                    sl = slice(32 * g, 32 * g + 32)
                    nc.tensor.matmul(
                        z1[sl, :], w_b, tiles[g][:, H:], start=st, stop=False,
                        tile_position=(0, 32 * g), skip_group_check=True,
                    )
                if b == 0:
                    # fold 0.1*lin_j into every row: lhsT has 0.1*W1 in
                    # all 128 columns, rhs = rightp (bf16).  Early: PSUM
                    # accumulation is order-independent.
                    nc.tensor.matmul(
                        z0[:], w1rep, rbT[:, :H], start=False, stop=False,
                        skip_group_check=True,
                    )
                    nc.tensor.matmul(
                        z1[:], w1rep, rbT[:, H:], start=False, stop=False,
                        skip_group_check=True,
                    )

            # ---- epilogue (pipelined per PSUM bank) ------------------
            rfs = big.tile([D, N], f32)
            prod = big.tile([D, N], f32)
            outc2 = const.tile([IPC, 2], f32)
            for h, zb in ((0, z0), (1, z1)):
                sl = slice(h * H, (h + 1) * H)
                nc.scalar.activation(
                    rfs[:, sl], zb[:], AF.Sigmoid, bias=biasvec[:], scale=1.0
                )
                nc.vector.tensor_tensor(
                    prod[:, sl], rfs[:, sl], cfsrep[:, sl], A_OP.mult
                )
                nc.vector.tensor_reduce(
                    outc2[:, h : h + 1],
                    prod[:, sl],
                    axis=mybir.AxisListType.X,
                    op=A_OP.max,
                )
            outc = const.tile([IPC, 1], f32)
            nc.vector.tensor_tensor(
                outc[:], outc2[:, 0:1], outc2[:, 1:2], A_OP.max
            )
            nc.gpsimd.dma_start(d_out[:], outc[:])

    return nc


def _host_prep(anon_e_emb, e_table, c_emb, r_emb, W0, b0, W1, b1):
    f = np.float32
    bft = ml_dtypes.bfloat16
    anon_e_emb = np.asarray(anon_e_emb, f)
    e_table = np.asarray(e_table, f)
    c_emb = np.asarray(c_emb, f)
    r_emb = np.asarray(r_emb, f)
    W0 = np.asarray(W0, f)
    b0 = np.asarray(b0, f)
    W1 = np.asarray(W1, f)
    b1 = np.asarray(b1, f)

    e_all = np.concatenate([e_table, anon_e_emb], axis=0)  # [N, D]
    e_allT = np.ascontiguousarray(e_all.T)  # [D, N]

    cols = np.zeros((D, 8), f)
    cols[:, 0] = W1
    cols[:, 2] = b0
    cols[:, 3] = r_emb
    cols[:, 4] = c_emb

    w0T = np.concatenate([W0[:, :D].T, W0[:, D:].T], axis=1)  # [wlT | wrT]

    bf_pack = np.zeros((D, _BF_COLS), bft)
    bf_pack[:, :N] = e_allT.astype(bft)
    bf_pack[:, N : N + D] = np.tile((0.1 * W1).astype(bft)[:, None], (1, D))
    # sbh: one-hot window buffer, col 128 = 0.9*W1; last col = 0.1*W1
    # (used as the lin_j column for the c-branch).
    sbh = np.zeros((D, 2 * D), bft)
    sbh[:, D] = (0.9 * W1).astype(bft)
    sbh[:, 2 * D - 1] = (0.1 * W1).astype(bft)
    bf_pack[:, N + D : N + 3 * D] = sbh
    bf_pack[:, N + 3 * D :] = W0[:, D:].T.astype(bft)

    rows = np.ones((1, D), f)
    b1f = float(b1[0])

    in_maps = []
    for c in range(NCORES):
        fp_pack = np.zeros((D, _FP_COLS), f)
        fp_pack[:, 0:8] = cols
        fp_pack[:, 8 : 8 + IPC] = e_allT[:, c * IPC : (c + 1) * IPC]
        fp_pack[:, 8 + IPC :] = w0T
        in_maps.append({"fp_pack": fp_pack, "bf_pack": bf_pack, "rows": rows})
    return in_maps, b1f


def _install_ntff_shim():
    """Provide antenv.axon_hooks (missing in this image) so that
    run_bass_kernel_spmd(trace=True) can collect NTFF profiles."""
    import sys
    import types

    if "antenv.axon_hooks" in sys.modules:
        return
    try:
        import antenv
        from trn_agent_boot.trn_boot import _ntff_profile_via_ctypes
    except ImportError:
        return
    mod = types.ModuleType("antenv.axon_hooks")
    state = {"hook": None}
    mod.set_axon_ntff_profile_hook = lambda h: state.__setitem__("hook", h)
    mod.get_axon_ntff_profile_hook = lambda: state["hook"]
    sys.modules["antenv.axon_hooks"] = mod
    antenv.axon_hooks = mod
    try:
        mod.set_axon_ntff_profile_hook(
            _ntff_profile_via_ctypes("/opt/axon/libaxon_pjrt.so")
        )
    except Exception:
        pass


def kernel_ex(inputs: dict, trace: bool = False):
    """Run on 8 NeuronCores; returns (out [N] float32, BassKernelResults)."""
    from concourse.bass_utils import run_bass_kernel_spmd

    if trace:
        _install_ntff_shim()

    in_maps, b1f = _host_prep(**inputs)
    key = (round(b1f, 10),)
    nc = _PROGRAM_CACHE.get(key)
    if nc is None:
        nc = _build_program(b1f)
        nc.finalize()
        _PROGRAM_CACHE[key] = nc

    res = run_bass_kernel_spmd(
        nc, in_maps, core_ids=list(range(NCORES)), trace=trace
    )
    out = np.concatenate(
        [
            np.asarray(res.results[c]["out"], np.float32).reshape(IPC)
            for c in range(NCORES)
        ]
    )
    return out, res


def kernel(**inputs) -> np.ndarray:
    out, _ = kernel_ex(inputs, trace=False)
    return out
